# revision 45
# baseline (speedup 1.0000x reference)
"""Multi-head cross-attention Trainium2 kernel (8 NeuronCores, SPMD).

Problem: nn_MultiHeadCrossAttention_31791347925263
  x:[4,2048,768], y:[4,2048,768], 12 heads x 64, fp32.
  out = softmax((x Wq^T)(y Wk^T)^T / 8 + mask) (y Wv^T) Wo^T   (+ zero biases)

Sharding: 8 cores = (batch b in 0..3) x (query half in 0..1).  Each core
computes the full attention for its 1024 query rows against all 2048 keys
of its batch.  No collectives; outputs concatenate.

Per-core dataflow (QK/qT-proj/oproj matmuls in f32r; kT/v' projections and
PV in bf16 -- y, Wk, Wv are converted to bf16 on the host, halving their
DMA and enabling fast weight load):
  kT  = WkT-blocks^T-matmul yT      -> [768(k-dim), 2048(sk)]  f32r
  v'  = yT-blocks^T-matmul WvT      -> [2048(sk), 12*128] bf16, per head
        [ones | 63 pad | 64 v]: ones col FIRST puts the softmax denominator
        on PSUM partition 0 (partition_broadcast sources p0); v cols start
        at partition 64 (engine APs must start 32-aligned)
  qT  = WqT-blocks^T-matmul xT      -> [768(q-dim), 1024(sq)]  f32r
  attention, per head pair, per sk-block (128 keys):
      S^T = kT-block^T-matmul qT    -> PSUM [128, 1024]
      P~  = exp(S^T/8): one head on ACT (exact exp), the other on the DVE
            as bitcast_bf16(i16(x*A + B)) (Schraudolph; softmax self-
            normalization cancels the common-mode error; ~0.9% end-to-end)
      valT_h += v'[skb,h]^T-matmul P~  -> PSUM [128, 1024], row 0 = denom
  norm (deferred into the NEXT pair's first skbs so its copies never sit in
  front of that pair's exps in an engine FIFO):
      vals = copy(valT)  [ACT/DVE split]; 1/den via int-magic seed + one
      Newton step on GPSIMD (idle engine; "int" ALU ops run in fp32
      semantics -- low-bit noise is irrelevant); partition_broadcast;
      vnorm = vals * bcast  [gpsimd; last pair: DVE + PE rank-1 broadcast]
  o[sqb] = vnorm-blocks^T-matmul WoT -> [1024, 768] -> DMA out
      (sqb0/1 accumulate pairs 0-4 BEFORE the last pair's norm lands)

Scheduling notes (the measured lessons; 412us -> ~385us):
  * The PE must stream gaplessly: one >3.4us PE-idle window rethrottles the
    HAM clock gate to 1.2 GHz and it tends to STICK there (cayman bug), so
    warm (2.4 GHz) is only self-sustaining while the per-skb exp+norm
    consumers stay faster than the PE.  Both exp engines together handle
    ~2.6us/skb vs the PE's ~1.9us/skb.
  * PE FIFO is software-pipelined: ... QK(n+1), PV(n) ... so PV's wait for
    exp(n) is covered by QK(n+1), and 3 QKs start each pair.
  * GPSIMD std<->ext library alternations (tensor ops vs broadcast) cost
    ~6-10us IRAM reloads: both heads' NR chains run, then both broadcasts.
  * PSUM: 2x [128,1024] score slots + 2x [128,1024] PV accumulators = all
    8 banks; matmul out <= 512 fp32 cols (one bank) per instruction.
"""

import numpy as np

B, S, D = 4, 2048, 768
H, Dh = 12, 64
SQ = S // 2          # queries per core
N_CORES = 8
DB = D // 128        # 6 d_model blocks
SKB = S // 128       # 16 key blocks
SQB = SQ // 128      # 8 query blocks per core
VPW = H * 128        # 128 cols/head: [ones | 63 pad | 64 v] (64-aligned v)

_cache = {}


# DVE Schraudolph exp: P~ = bitcast_bf16(i16(x*A + B)), written as int16
# bit patterns into a bf16 tile (the BIR verifier forbids int->f32r puns
# but not int16->bf16).  A folds the 1/8 score scale.
SCHRAUD_A = float(0.125 * 128.0 / np.log(2.0))
SCHRAUD_B = float(127 * 128 - 7.42)
RECIP_MAGIC = 0x7EF311C3


def _build_nc():
    import concourse.mybir as mybir
    import concourse.tile as tile
    from concourse import bacc

    f32 = mybir.dt.float32
    f32r = mybir.dt.float32r
    bf16 = mybir.dt.bfloat16
    i16 = mybir.dt.int16
    EXP = mybir.ActivationFunctionType.Exp
    MUL = mybir.AluOpType.mult
    ADD = mybir.AluOpType.add
    i32 = mybir.dt.int32

    nc = bacc.Bacc("TRN2", target_bir_lowering=False)
    xT = nc.dram_tensor("xT", [D, SQ], f32, kind="ExternalInput")
    yT = nc.dram_tensor("yT", [D, S], bf16, kind="ExternalInput")
    WqT = nc.dram_tensor("WqT", [D, D], f32, kind="ExternalInput")
    WkT = nc.dram_tensor("WkT", [D, D], bf16, kind="ExternalInput")
    WvT = nc.dram_tensor("WvT", [D, D], bf16, kind="ExternalInput")
    WoT = nc.dram_tensor("WoT", [D, D], f32, kind="ExternalInput")
    out = nc.dram_tensor("out", [SQ, D], f32, kind="ExternalOutput")

    with tile.TileContext(nc) as tc:
        with tc.tile_pool(name="persist", bufs=1) as pp, \
             tc.tile_pool(name="mmps", bufs=2, space="PSUM") as mm_ps, \
             tc.tile_pool(name="vtp", bufs=2, space="PSUM") as vt_ps:

            def mm_tile(cols):
                return mm_ps.tile([128, cols], f32, name="mmps", tag="mmps",
                                  padded_shape=[128, SQ])

            kT = [pp.tile([128, S], f32r, name=f"kT{i}") for i in range(DB)]
            vp = [pp.tile([128, VPW], bf16, name=f"vp{i}") for i in range(SKB)]
            qT = [pp.tile([128, SQ], f32r, name=f"qT{i}") for i in range(DB)]
            ones64 = pp.tile([1, 64], bf16, name="ones64")
            nc.vector.memset(ones64[:, :], 1.0)
            vnorm = qT  # valnorm overwrites qT (same shape; see docstring)

            ld_x_cm = tc.tile_pool(name="ld_x", bufs=1)
            ld_x = ld_x_cm.__enter__()
            xTs = [ld_x.tile([128, SQ], f32r, name=f"xTs{i}")
                   for i in range(DB)]
            wqTs = [ld_x.tile([128, D], f32r, name=f"wqTs{i}")
                    for i in range(DB)]

            with tc.tile_pool(name="ld_y", bufs=1) as ld_y:
                yTs = [ld_y.tile([128, S], bf16, name=f"yTs{i}")
                       for i in range(DB)]

                # ---- kT projection: kT[ob] = (WkT col-block)^T @ yT ----
                with tc.tile_pool(name="ld_wk", bufs=1) as ld_wk:
                    wkTs = [ld_wk.tile([128, D], bf16, name=f"wkTs{i}")
                            for i in range(DB)]
                    # interleave so the first kT matmul's operands
                    # (wk block kb + y chunk 0) land earliest
                    for i in range(DB):
                        nc.sync.dma_start(
                            out=wkTs[i],
                            in_=WkT[i * 128:(i + 1) * 128, :])
                        nc.sync.dma_start(
                            out=yTs[i][:, 0:512],
                            in_=yT[i * 128:(i + 1) * 128, 0:512])
                    for c4 in range(1, 4):
                        for i in range(DB):
                            nc.sync.dma_start(
                                out=yTs[i][:, c4 * 512:(c4 + 1) * 512],
                                in_=yT[i * 128:(i + 1) * 128,
                                       c4 * 512:(c4 + 1) * 512])
                    wvTs = [ld_y.tile([128, D], bf16, name=f"wvTs{i}")
                            for i in range(DB)]
                    for i in range(DB):
                        nc.sync.dma_start(
                            out=wvTs[i],
                            in_=WvT[i * 128:(i + 1) * 128, :])
                    # x/Wq loads issued up front: ld_x is a disjoint pool
                    # alive from the start, so these DMAs overlap kT/v' math
                    # instead of waiting for the wk pool SBUF to free up
                    for i in range(DB):
                        nc.sync.dma_start(
                            out=wqTs[i],
                            in_=WqT[i * 128:(i + 1) * 128, :].bitcast(f32r))
                    for c2 in range(2):
                        for i in range(DB):
                            nc.sync.dma_start(
                                out=xTs[i][:, c2 * 512:(c2 + 1) * 512],
                                in_=xT[i * 128:(i + 1) * 128,
                                       c2 * 512:(c2 + 1) * 512].bitcast(f32r))
                    # nc4 outer: the first 6 groups need only yT column
                    # chunk 0, so compute starts while chunks 1-3 stream in
                    for nc4 in range(4):
                        for ob in range(DB):
                            ps = mm_tile(512)
                            for kb in range(DB):
                                nc.tensor.matmul(
                                    ps[:, :],
                                    wkTs[kb][:, ob * 128:(ob + 1) * 128],
                                    yTs[kb][:, nc4 * 512:(nc4 + 1) * 512],
                                    start=(kb == 0), stop=(kb == DB - 1))
                            nc.scalar.copy(
                                kT[ob][:, nc4 * 512:(nc4 + 1) * 512],
                                ps[:, :])

                # ---- v' projection: v[skb] = (yT blk)^T @ WvT ----
                if True:
                    for skb in range(SKB):
                        vps3 = vp[skb].rearrange("p (h c) -> p h c", c=128)
                        nc.vector.memset(vps3[:, :, 0:64], 0.0)
                        nc.vector.memset(vps3[:, :, 0], 1.0)
                        for nc2 in range(2):
                            n0, n1 = nc2 * 512, min(D, (nc2 + 1) * 512)
                            ps = mm_tile(512)
                            for kb in range(DB):
                                nc.tensor.matmul(
                                    ps[:, 0:n1 - n0],
                                    yTs[kb][:, skb * 128:(skb + 1) * 128],
                                    wvTs[kb][:, n0:n1],
                                    start=(kb == 0), stop=(kb == DB - 1))
                            # contiguous v-cols -> 65-strided layout
                            src = ps[:, 0:n1 - n0].rearrange(
                                "p (h c) -> p h c", c=Dh)
                            dst = vps3[:, nc2 * 8:nc2 * 8 + (n1 - n0) // Dh,
                                       64:128]
                            nc.scalar.copy(dst, src)

            # ---- qT projection (x/Wq already resident) ----
            if True:
                for nc2 in range(2):
                    for ob in range(DB):
                        ps = mm_tile(512)
                        for kb in range(DB):
                            nc.tensor.matmul(
                                ps[:, :],
                                wqTs[kb][:, ob * 128:(ob + 1) * 128],
                                xTs[kb][:, nc2 * 512:(nc2 + 1) * 512],
                                start=(kb == 0), stop=(kb == DB - 1))
                        nc.vector.tensor_copy(
                            qT[ob][:, nc2 * 512:(nc2 + 1) * 512], ps[:, :])

            ld_x_cm.__exit__(None, None, None)

            # ---- attention ----
            with tc.tile_pool(name="late", bufs=1) as lp:
                woT = [lp.tile([128, D], f32r, name=f"woT{i}")
                       for i in range(DB)]
                for i in range(DB):
                    nc.sync.dma_start(
                        out=woT[i],
                        in_=WoT[i * 128:(i + 1) * 128, :].bitcast(f32r))

                with tc.tile_pool(name="psb", bufs=5) as p_pool, \
                     tc.tile_pool(name="nrm", bufs=2) as nrm_pool:

                    def make_norm(hb, vt0, vt1):
                        # Normalization for pair hb, emitted DEFERRED (during
                        # the next pair's first skbs) so the vals copies never
                        # sit in front of that pair's exps in an engine FIFO
                        # (a >3.4us PE gap rethrottles HAM and the clock tends
                        # to stick at 1.2 GHz).  Phases: copies (split ACT/
                        # DVE), both NR chains, both broadcasts ADJACENT (each
                        # gpsimd std<->ext library alternation costs ~6-10us
                        # of IRAM reload), then both muls.
                        def go():
                            last = hb == H // 2 - 1
                            vals01 = []
                            for h, vt in ((2 * hb, vt0), (2 * hb + 1, vt1)):
                                vals = nrm_pool.tile([128, SQ], f32,
                                                     name="vals")
                                if h % 2 == 0:
                                    nc.scalar.copy(vals[:, :], vt[:, :])
                                else:
                                    nc.vector.tensor_copy(vals[:, :],
                                                          vt[:, :])
                                vals01.append(vals)
                            eng = nc.vector if last else nc.gpsimd
                            recs = []
                            for vals in vals01:
                                # 1/den: int-magic seed bitcast(magic-bits)
                                # = i*(-1)+magic ("int" ALU ops run in float
                                # semantics; low-bit noise irrelevant), one
                                # Newton step r1 = r0*(2-d*r0) -> ~0.3% err.
                                # den sits on partition 0 (ones col FIRST in
                                # vp): partition_broadcast sources p0.
                                den = vals[0:1, :]
                                sd = nrm_pool.tile([1, SQ], f32, name="sd")
                                eng.tensor_scalar(
                                    sd.bitcast(i32)[:, :], den.bitcast(i32),
                                    -1, RECIP_MAGIC, MUL, ADD)
                                t2 = nrm_pool.tile([1, SQ], f32, name="t2")
                                eng.tensor_tensor(t2[:, :], den, sd[:, :],
                                                  MUL)
                                eng.tensor_scalar(t2[:, :], t2[:, :],
                                                  -1.0, 2.0, MUL, ADD)
                                eng.tensor_tensor(t2[:, :], t2[:, :],
                                                  sd[:, :], MUL)
                                recs.append(t2)
                            if last:
                                # PE rank-1 broadcast (ones x rec) into the
                                # now-idle score slots: keeps gpsimd (and its
                                # ~6-10us std<->ext library reloads) off the
                                # critical tail entirely
                                rb16s = []
                                for t2 in recs:
                                    rb16 = nrm_pool.tile([1, SQ], bf16,
                                                         name="rb16")
                                    nc.vector.tensor_copy(rb16[:, :],
                                                          t2[:, :])
                                    rb16s.append(rb16)
                                for i, (vals, rb16) in enumerate(
                                        zip(vals01, rb16s)):
                                    rps = vt_ps.tile(
                                        [128, SQ], f32, name="valT",
                                        tag="valT",
                                        padded_shape=[128, SQ])
                                    for j in range(2):
                                        nc.tensor.matmul(
                                            rps[64:128,
                                                j * 512:(j + 1) * 512],
                                            ones64[:, :],
                                            rb16[:, j * 512:(j + 1) * 512],
                                            start=True, stop=True)
                                    nc.vector.tensor_tensor(
                                        vnorm[hb][i * 64:i * 64 + 64, :],
                                        vals[64:128, :], rps[64:128, :], MUL)
                            else:
                                rbcs = []
                                for t2 in recs:
                                    rbc = nrm_pool.tile([128, SQ], f32,
                                                        name="rbc")
                                    nc.gpsimd.partition_broadcast(
                                        rbc[:, :], t2[0:1, :])
                                    rbcs.append(rbc)
                                for i, (vals, rbc) in enumerate(
                                        zip(vals01, rbcs)):
                                    r0_ = i * 64
                                    nc.gpsimd.tensor_tensor(
                                        vnorm[hb][r0_:r0_ + 64, :],
                                        vals[64:128, :], rbc[64:128, :], MUL)
                        return go

                    pending_norm = None
                    for hb in range(H // 2):
                        h0, h1 = 2 * hb, 2 * hb + 1

                        def qk_exp(skb):
                            st0 = mm_tile(SQ)
                            st1 = mm_tile(SQ)
                            for r0, st in ((0, st0), (64, st1)):
                                for j in range(2):
                                    nc.tensor.matmul(
                                        st[:, j * 512:(j + 1) * 512],
                                        kT[hb][r0:r0 + 64,
                                               skb * 128:(skb + 1) * 128],
                                        qT[hb][r0:r0 + 64,
                                               j * 512:(j + 1) * 512],
                                        start=True, stop=True)
                            pt0 = p_pool.tile([128, SQ], bf16, name="pTb")
                            pt1 = p_pool.tile([128, SQ], bf16, name="pTb")
                            # One head to ACT (exact exp), one to DVE
                            # (Schraudolph), alternating per skb.  Each
                            # engine then needs ~1.3us per skb < the PE's
                            # ~1.9us -> the PE streams gaplessly, keeping the
                            # HAM clock-gate at 8/8 (warm is only self-
                            # sustaining if the PE never waits: once HAM
                            # rethrottles it tends to stick cold).
                            if hb == 0 and skb < 3:
                                # ACT may still be draining projection
                                # copies right at the transition
                                pairs = ((None, None), (pt0, st0), (pt1, st1))
                            elif skb % 2 == 0:
                                pairs = ((pt0, st0), (pt1, st1), (None, None))
                            else:
                                pairs = ((pt1, st1), (pt0, st0), (None, None))
                            (a_pt, a_st), (d1_pt, d1_st), (d2_pt, d2_st) = \
                                pairs
                            if a_pt is not None:
                                nc.scalar.activation(a_pt[:, :], a_st[:, :],
                                                     EXP, scale=0.125)
                            for pt, st in ((d1_pt, d1_st), (d2_pt, d2_st)):
                                if pt is not None:
                                    nc.vector.tensor_scalar(
                                        pt.bitcast(i16)[:, :], st[:, :],
                                        SCHRAUD_A, SCHRAUD_B, MUL, ADD)
                            return (pt0, pt1)

                        def pv(skb, pts, vt0, vt1):
                            pt0, pt1 = pts
                            for h, vt, pt in ((h0, vt0, pt0), (h1, vt1, pt1)):
                                for j in range(2):
                                    nc.tensor.matmul(
                                        vt[:, j * 512:(j + 1) * 512],
                                        vp[skb][:, h * 128:(h + 1) * 128],
                                        pt[:, j * 512:(j + 1) * 512],
                                        start=(skb == 0),
                                        stop=(skb == SKB - 1))

                        # prologue: QK+exp of skbs 0-1, then the previous
                        # pair's deferred norm, then allocate this pair's vt
                        # tiles (allocation after the norm keeps the PSUM-
                        # slot dependency order correct).  Two skbs of QK
                        # give the PE enough work to cover the vals copies
                        # that free the vt slots.
                        # One-stage software pipeline in the PE FIFO:
                        # ... QK(n+1), PV(n), QK(n+2), PV(n+1) ... -- while
                        # PV(n) waits for exp(n), the PE is already running
                        # QK(n+1) instead of idling behind it in the queue.
                        # 3 QKs up front give the PE work to cover the
                        # deferred norm's copies at the pair boundary.
                        plg = [qk_exp(0), qk_exp(1), qk_exp(2)]
                        if pending_norm is not None:
                            pending_norm()
                        vt0 = vt_ps.tile([128, SQ], f32, name="valT")
                        vt1 = vt_ps.tile([128, SQ], f32, name="valT")
                        pv(0, plg[0], vt0, vt1)
                        pv(1, plg[1], vt0, vt1)
                        prev = plg[2]
                        for skb in range(3, SKB):
                            cur = qk_exp(skb)
                            pv(skb - 1, prev, vt0, vt1)
                            prev = cur
                        pv(SKB - 1, prev, vt0, vt1)
                        pending_norm = make_norm(hb, vt0, vt1)

                    # ---- output projection ----
                    # phase 1: open sqb0/sqb1 accumulators over kb 0..4 (only
                    # needs pairs 0-4) so the PE chews on them while the last
                    # pair's norm chain (emitted next) runs on ACT/DVE
                    def oproj_mms(op, sqb, kbs, first, final):
                        for nc2 in range(2):
                            n0, n1 = nc2 * 512, min(D, (nc2 + 1) * 512)
                            for kb in kbs:
                                nc.tensor.matmul(
                                    op[:, n0:n1],
                                    vnorm[kb][:, sqb * 128:(sqb + 1) * 128],
                                    woT[kb][:, n0:n1],
                                    start=(kb == 0 and first),
                                    stop=(kb == DB - 1 and final))

                    with tc.tile_pool(name="osb", bufs=3) as o_pool:
                        op01 = []
                        for sqb in range(2):
                            op = mm_tile(D)
                            oproj_mms(op, sqb, range(DB - 1), True, False)
                            op01.append(op)
                        pending_norm()
                        for sqb in range(SQB):
                            if sqb < 2:
                                op = op01[sqb]
                                oproj_mms(op, sqb, [DB - 1], False, True)
                            elif sqb % 2 == 0:
                                op = mm_tile(D)
                                oproj_mms(op, sqb, range(DB), True, True)
                            else:
                                op = vt_ps.tile([128, D], f32, name="valT",
                                                tag="valT",
                                                padded_shape=[128, SQ])
                                oproj_mms(op, sqb, range(DB), True, True)
                            ot = o_pool.tile([128, D], f32, name="osb")
                            nc.vector.tensor_copy(ot[:, :], op[:, :])
                            nc.sync.dma_start(
                                out=out[sqb * 128:(sqb + 1) * 128, :],
                                in_=ot[:, :])

    nc.compile()
    return nc


def _get_nc():
    if "nc" not in _cache:
        _cache["nc"] = _build_nc()
    return _cache["nc"]


def _host_fallback(x, y, mask, Wq, bq, Wkv, bkv, Wo, bo):
    Bb, Ss, _ = x.shape
    q = x @ Wq.T + bq
    kv = y @ Wkv.T + bkv
    q = q.reshape(Bb, Ss, H, Dh).transpose(0, 2, 1, 3)
    kv = kv.reshape(Bb, Ss, H, 2 * Dh).transpose(0, 2, 1, 3)
    k, v = kv[..., :Dh], kv[..., Dh:]
    scaled = np.einsum("bhqd,bhkd->bhqk", q, k) / np.sqrt(np.float32(Dh))
    scaled = scaled + mask
    scaled -= scaled.max(axis=-1, keepdims=True)
    e = np.exp(scaled)
    attn = e / e.sum(axis=-1, keepdims=True)
    values = np.einsum("bhqk,bhkd->bhqd", attn, v)
    values = values.transpose(0, 2, 1, 3).reshape(Bb, Ss, H * Dh)
    return (values @ Wo.T + bo).astype(np.float32)


def _run(inputs, trace=False, trace_cores=None):
    """Returns (full_output, BassKernelResults)."""
    from concourse.bass_utils import run_bass_kernel_spmd

    x = np.ascontiguousarray(np.asarray(inputs["x"], dtype=np.float32))
    y = np.ascontiguousarray(np.asarray(inputs["y"], dtype=np.float32))
    Wq = np.asarray(inputs["Wq"], dtype=np.float32)
    Wkv = np.asarray(inputs["Wkv"], dtype=np.float32)
    Wo = np.asarray(inputs["Wo"], dtype=np.float32)

    # Reference reshapes kv to [B,S,H,2*Dh]: per head, rows h*128..h*128+63 of
    # Wkv are the k-projection, rows h*128+64..h*128+127 the v-projection.
    k_rows = np.concatenate([np.arange(h * 128, h * 128 + Dh) for h in range(H)])
    v_rows = np.concatenate([np.arange(h * 128 + Dh, (h + 1) * 128)
                             for h in range(H)])
    import ml_dtypes
    bf16 = ml_dtypes.bfloat16
    WqT = np.ascontiguousarray(Wq.T)
    WkT = np.ascontiguousarray(Wkv[k_rows].T.astype(bf16))
    WvT = np.ascontiguousarray(Wkv[v_rows].T.astype(bf16))
    WoT = np.ascontiguousarray(Wo.T)

    in_maps = []
    for c in range(N_CORES):
        b, half = c // 2, c % 2
        xTc = np.ascontiguousarray(x[b, half * SQ:(half + 1) * SQ, :].T)
        yTb = np.ascontiguousarray(y[b].T.astype(bf16))
        in_maps.append({"xT": xTc, "yT": yTb, "WqT": WqT, "WkT": WkT,
                        "WvT": WvT, "WoT": WoT})

    nc = _get_nc()
    res = run_bass_kernel_spmd(nc, in_maps, core_ids=list(range(N_CORES)),
                               trace=trace, trace_cores=trace_cores)
    out = np.empty((B, S, D), dtype=np.float32)
    for c in range(N_CORES):
        b, half = c // 2, c % 2
        out[b, half * SQ:(half + 1) * SQ, :] = res.results[c]["out"]
    return out, res


def kernel(**inputs) -> np.ndarray:
    mask = np.asarray(inputs["mask"], dtype=np.float32)
    bq = np.asarray(inputs["bq"], dtype=np.float32)
    bkv = np.asarray(inputs["bkv"], dtype=np.float32)
    bo = np.asarray(inputs["bo"], dtype=np.float32)
    if mask.any() or bq.any() or bkv.any() or bo.any():
        # Device kernel hardcodes zero mask/biases; stay correct regardless.
        return _host_fallback(
            np.asarray(inputs["x"], dtype=np.float32),
            np.asarray(inputs["y"], dtype=np.float32),
            mask, np.asarray(inputs["Wq"], dtype=np.float32), bq,
            np.asarray(inputs["Wkv"], dtype=np.float32), bkv,
            np.asarray(inputs["Wo"], dtype=np.float32), bo)
    out, _ = _run(inputs)
    return out



# revision 46
# speedup vs baseline: 1.1845x; 1.1845x over previous
"""Multi-head cross-attention Trainium2 kernel (8 NeuronCores, SPMD).

Problem: nn_MultiHeadCrossAttention_31791347925263
  x:[4,2048,768], y:[4,2048,768], 12 heads x 64, fp32.
  out = softmax((x Wq^T)(y Wk^T)^T / 8 + mask) (y Wv^T) Wo^T   (+ zero biases)

Sharding: 8 cores = (batch b in 0..3) x (query half in 0..1).  Each core
computes the full attention for its 1024 query rows against all 2048 keys
of its batch.  No collectives; outputs concatenate.

Per-core dataflow (QK/qT-proj/oproj matmuls in f32r; kT/v' projections and
PV in bf16 -- y, Wk, Wv are converted to bf16 on the host, halving their
DMA and enabling fast weight load):
  kT  = WkT-blocks^T-matmul yT      -> [768(k-dim), 2048(sk)]  f32r
  v'  = yT-blocks^T-matmul WvT      -> [2048(sk), 12*128] bf16, per head
        [ones | 63 pad | 64 v]: ones col FIRST puts the softmax denominator
        on PSUM partition 0 (partition_broadcast sources p0); v cols start
        at partition 64 (engine APs must start 32-aligned)
  qT  = WqT-blocks^T-matmul xT      -> [768(q-dim), 1024(sq)]  f32r
  attention, per head pair, per sk-block (128 keys):
      S^T = kT-block^T-matmul qT    -> PSUM [128, 1024]
      P~  = exp(S^T/8): one head on ACT (exact exp), the other on the DVE
            as bitcast_bf16(i16(x*A + B)) (Schraudolph; softmax self-
            normalization cancels the common-mode error; ~0.9% end-to-end)
      valT_h += v'[skb,h]^T-matmul P~  -> PSUM [128, 1024], row 0 = denom
  norm (deferred into the NEXT pair's first skbs so its copies never sit in
  front of that pair's exps in an engine FIFO):
      vals = copy(valT)  [ACT/DVE split]; 1/den via int-magic seed + one
      Newton step on GPSIMD (idle engine; "int" ALU ops run in fp32
      semantics -- low-bit noise is irrelevant); partition_broadcast;
      vnorm = vals * bcast  [gpsimd; last pair: DVE + PE rank-1 broadcast]
  o[sqb] = vnorm-blocks^T-matmul WoT -> [1024, 768] -> DMA out
      (sqb0/1 accumulate pairs 0-4 BEFORE the last pair's norm lands)

Scheduling notes (the measured lessons; 412us -> ~385us):
  * The PE must stream gaplessly: one >3.4us PE-idle window rethrottles the
    HAM clock gate to 1.2 GHz and it tends to STICK there (cayman bug), so
    warm (2.4 GHz) is only self-sustaining while the per-skb exp+norm
    consumers stay faster than the PE.  Both exp engines together handle
    ~2.6us/skb vs the PE's ~1.9us/skb.
  * PE FIFO is software-pipelined: ... QK(n+1), PV(n) ... so PV's wait for
    exp(n) is covered by QK(n+1), and 3 QKs start each pair.
  * GPSIMD std<->ext library alternations (tensor ops vs broadcast) cost
    ~6-10us IRAM reloads: both heads' NR chains run, then both broadcasts.
  * PSUM: 2x [128,1024] score slots + 2x [128,1024] PV accumulators = all
    8 banks; matmul out <= 512 fp32 cols (one bank) per instruction.
"""

import numpy as np

B, S, D = 4, 2048, 768
H, Dh = 12, 64
SQ = S // 2          # queries per core
N_CORES = 8
DB = D // 128        # 6 d_model blocks
SKB = S // 128       # 16 key blocks
SQB = SQ // 128      # 8 query blocks per core
VPW = H * 128        # 128 cols/head: [ones | 63 pad | 64 v] (64-aligned v)

_cache = {}


# DVE Schraudolph exp: P~ = bitcast_bf16(i16(x*A + B)), written as int16
# bit patterns into a bf16 tile (the BIR verifier forbids int->f32r puns
# but not int16->bf16).  A folds the 1/8 score scale.
SCHRAUD_A = float(0.125 * 128.0 / np.log(2.0))
SCHRAUD_B = float(127 * 128 - 7.42)
RECIP_MAGIC = 0x7EF311C3


def _build_nc():
    import concourse.mybir as mybir
    import concourse.tile as tile
    from concourse import bacc

    f32 = mybir.dt.float32
    f32r = mybir.dt.float32r
    bf16 = mybir.dt.bfloat16
    i16 = mybir.dt.int16
    EXP = mybir.ActivationFunctionType.Exp
    MUL = mybir.AluOpType.mult
    ADD = mybir.AluOpType.add
    i32 = mybir.dt.int32

    nc = bacc.Bacc("TRN2", target_bir_lowering=False)
    xT = nc.dram_tensor("xT", [D, SQ], f32, kind="ExternalInput")
    yT = nc.dram_tensor("yT", [D, S], bf16, kind="ExternalInput")
    WqT = nc.dram_tensor("WqT", [D, D], f32, kind="ExternalInput")
    WkT = nc.dram_tensor("WkT", [D, D], bf16, kind="ExternalInput")
    WvT = nc.dram_tensor("WvT", [D, D], bf16, kind="ExternalInput")
    WoT = nc.dram_tensor("WoT", [D, D], bf16, kind="ExternalInput")
    out = nc.dram_tensor("out", [SQ, D], f32, kind="ExternalOutput")

    with tile.TileContext(nc) as tc:
        with tc.tile_pool(name="persist", bufs=1) as pp, \
             tc.tile_pool(name="mmps", bufs=2, space="PSUM") as mm_ps, \
             tc.tile_pool(name="vtp", bufs=2, space="PSUM") as vt_ps:

            def mm_tile(cols):
                return mm_ps.tile([128, cols], f32, name="mmps", tag="mmps",
                                  padded_shape=[128, SQ])

            kT = [pp.tile([128, S], f32r, name=f"kT{i}") for i in range(DB)]
            vp = [pp.tile([128, VPW], bf16, name=f"vp{i}") for i in range(SKB)]
            qT = [pp.tile([128, SQ], f32r, name=f"qT{i}") for i in range(DB)]
            ones64 = pp.tile([1, 64], bf16, name="ones64")
            nc.vector.memset(ones64[:, :], 1.0)


            ld_x_cm = tc.tile_pool(name="ld_x", bufs=1)
            ld_x = ld_x_cm.__enter__()
            xTs = [ld_x.tile([128, SQ], f32r, name=f"xTs{i}")
                   for i in range(DB)]
            wqTs = [ld_x.tile([128, D], f32r, name=f"wqTs{i}")
                    for i in range(DB)]

            with tc.tile_pool(name="ld_y", bufs=1) as ld_y:
                yTs = [ld_y.tile([128, S], bf16, name=f"yTs{i}")
                       for i in range(DB)]

                # ---- kT projection: kT[ob] = (WkT col-block)^T @ yT ----
                with tc.tile_pool(name="ld_wk", bufs=1) as ld_wk:
                    wkTs = [ld_wk.tile([128, D], bf16, name=f"wkTs{i}")
                            for i in range(DB)]
                    # interleave so the first kT matmul's operands
                    # (wk block kb + y chunk 0) land earliest
                    for i in range(DB):
                        nc.sync.dma_start(
                            out=wkTs[i],
                            in_=WkT[i * 128:(i + 1) * 128, :])
                        nc.sync.dma_start(
                            out=yTs[i][:, 0:512],
                            in_=yT[i * 128:(i + 1) * 128, 0:512])
                    for c4 in range(1, 4):
                        for i in range(DB):
                            nc.sync.dma_start(
                                out=yTs[i][:, c4 * 512:(c4 + 1) * 512],
                                in_=yT[i * 128:(i + 1) * 128,
                                       c4 * 512:(c4 + 1) * 512])
                    wvTs = [ld_y.tile([128, D], bf16, name=f"wvTs{i}")
                            for i in range(DB)]
                    for i in range(DB):
                        nc.sync.dma_start(
                            out=wvTs[i],
                            in_=WvT[i * 128:(i + 1) * 128, :])
                    # x/Wq loads issued up front: ld_x is a disjoint pool
                    # alive from the start, so these DMAs overlap kT/v' math
                    # instead of waiting for the wk pool SBUF to free up
                    for i in range(DB):
                        nc.sync.dma_start(
                            out=wqTs[i],
                            in_=WqT[i * 128:(i + 1) * 128, :].bitcast(f32r))
                    for c2 in range(2):
                        for i in range(DB):
                            nc.sync.dma_start(
                                out=xTs[i][:, c2 * 512:(c2 + 1) * 512],
                                in_=xT[i * 128:(i + 1) * 128,
                                       c2 * 512:(c2 + 1) * 512].bitcast(f32r))
                    # nc4 outer: the first 6 groups need only yT column
                    # chunk 0, so compute starts while chunks 1-3 stream in
                    for nc4 in range(4):
                        for ob in range(DB):
                            ps = mm_tile(512)
                            for kb in range(DB):
                                nc.tensor.matmul(
                                    ps[:, :],
                                    wkTs[kb][:, ob * 128:(ob + 1) * 128],
                                    yTs[kb][:, nc4 * 512:(nc4 + 1) * 512],
                                    start=(kb == 0), stop=(kb == DB - 1))
                            nc.scalar.copy(
                                kT[ob][:, nc4 * 512:(nc4 + 1) * 512],
                                ps[:, :])

                # ---- v' projection: v[skb] = (yT blk)^T @ WvT ----
                if True:
                    for skb in range(SKB):
                        vps3 = vp[skb].rearrange("p (h c) -> p h c", c=128)
                        nc.vector.memset(vps3[:, :, 0:64], 0.0)
                        nc.vector.memset(vps3[:, :, 0], 1.0)
                        for nc2 in range(2):
                            n0, n1 = nc2 * 512, min(D, (nc2 + 1) * 512)
                            ps = mm_tile(512)
                            for kb in range(DB):
                                nc.tensor.matmul(
                                    ps[:, 0:n1 - n0],
                                    yTs[kb][:, skb * 128:(skb + 1) * 128],
                                    wvTs[kb][:, n0:n1],
                                    start=(kb == 0), stop=(kb == DB - 1))
                            # contiguous v-cols -> 65-strided layout
                            src = ps[:, 0:n1 - n0].rearrange(
                                "p (h c) -> p h c", c=Dh)
                            dst = vps3[:, nc2 * 8:nc2 * 8 + (n1 - n0) // Dh,
                                       64:128]
                            nc.scalar.copy(dst, src)

            # ---- qT projection (x/Wq already resident) ----
            if True:
                for nc2 in range(2):
                    for ob in range(DB):
                        ps = mm_tile(512)
                        for kb in range(DB):
                            nc.tensor.matmul(
                                ps[:, :],
                                wqTs[kb][:, ob * 128:(ob + 1) * 128],
                                xTs[kb][:, nc2 * 512:(nc2 + 1) * 512],
                                start=(kb == 0), stop=(kb == DB - 1))
                        nc.vector.tensor_copy(
                            qT[ob][:, nc2 * 512:(nc2 + 1) * 512], ps[:, :])

            ld_x_cm.__exit__(None, None, None)

            # ---- attention ----
            with tc.tile_pool(name="late", bufs=1) as lp:
                woT = [lp.tile([128, D], bf16, name=f"woT{i}")
                       for i in range(DB)]
                for i in range(DB):
                    nc.sync.dma_start(
                        out=woT[i],
                        in_=WoT[i * 128:(i + 1) * 128, :])
                # bf16 oproj: halves the (tail-exposed) vnorm/woT weight
                # loads; vnorm no longer aliases qT
                vnorm = [lp.tile([128, SQ], bf16, name=f"vn{i}")
                         for i in range(DB)]

                with tc.tile_pool(name="psb", bufs=5) as p_pool, \
                     tc.tile_pool(name="nrm", bufs=2) as nrm_pool:

                    def make_norm(hb, vt0, vt1):
                        # Normalization for pair hb, emitted DEFERRED (during
                        # the next pair's first skbs) so the vals copies never
                        # sit in front of that pair's exps in an engine FIFO
                        # (a >3.4us PE gap rethrottles HAM and the clock tends
                        # to stick at 1.2 GHz).  Phases: copies (split ACT/
                        # DVE), both NR chains, both broadcasts ADJACENT (each
                        # gpsimd std<->ext library alternation costs ~6-10us
                        # of IRAM reload), then both muls.
                        def go():
                            last = hb == H // 2 - 1
                            vals01 = []
                            for h, vt in ((2 * hb, vt0), (2 * hb + 1, vt1)):
                                vals = nrm_pool.tile([128, SQ], f32,
                                                     name="vals")
                                if h % 2 == 0:
                                    nc.scalar.copy(vals[:, :], vt[:, :])
                                else:
                                    nc.vector.tensor_copy(vals[:, :],
                                                          vt[:, :])
                                vals01.append(vals)
                            eng = nc.vector if last else nc.gpsimd
                            recs = []
                            for vals in vals01:
                                # 1/den: int-magic seed bitcast(magic-bits)
                                # = i*(-1)+magic ("int" ALU ops run in float
                                # semantics; low-bit noise irrelevant), one
                                # Newton step r1 = r0*(2-d*r0) -> ~0.3% err.
                                # den sits on partition 0 (ones col FIRST in
                                # vp): partition_broadcast sources p0.
                                den = vals[0:1, :]
                                sd = nrm_pool.tile([1, SQ], f32, name="sd")
                                eng.tensor_scalar(
                                    sd.bitcast(i32)[:, :], den.bitcast(i32),
                                    -1, RECIP_MAGIC, MUL, ADD)
                                t2 = nrm_pool.tile([1, SQ], f32, name="t2")
                                eng.tensor_tensor(t2[:, :], den, sd[:, :],
                                                  MUL)
                                eng.tensor_scalar(t2[:, :], t2[:, :],
                                                  -1.0, 2.0, MUL, ADD)
                                eng.tensor_tensor(t2[:, :], t2[:, :],
                                                  sd[:, :], MUL)
                                recs.append(t2)
                            if last:
                                # PE rank-1 broadcast (ones x rec) into the
                                # now-idle score slots: keeps gpsimd (and its
                                # ~6-10us std<->ext library reloads) off the
                                # critical tail entirely
                                rb16s = []
                                for t2 in recs:
                                    rb16 = nrm_pool.tile([1, SQ], bf16,
                                                         name="rb16")
                                    nc.vector.tensor_copy(rb16[:, :],
                                                          t2[:, :])
                                    rb16s.append(rb16)
                                for i, (vals, rb16) in enumerate(
                                        zip(vals01, rb16s)):
                                    rps = vt_ps.tile(
                                        [128, SQ], f32, name="valT",
                                        tag="valT",
                                        padded_shape=[128, SQ])
                                    for j in range(2):
                                        nc.tensor.matmul(
                                            rps[64:128,
                                                j * 512:(j + 1) * 512],
                                            ones64[:, :],
                                            rb16[:, j * 512:(j + 1) * 512],
                                            start=True, stop=True)
                                    nc.vector.tensor_tensor(
                                        vnorm[hb][i * 64:i * 64 + 64, :],
                                        vals[64:128, :], rps[64:128, :], MUL)
                            else:
                                rbcs = []
                                for t2 in recs:
                                    rbc = nrm_pool.tile([128, SQ], f32,
                                                        name="rbc")
                                    nc.gpsimd.partition_broadcast(
                                        rbc[:, :], t2[0:1, :])
                                    rbcs.append(rbc)
                                for i, (vals, rbc) in enumerate(
                                        zip(vals01, rbcs)):
                                    r0_ = i * 64
                                    nc.gpsimd.tensor_tensor(
                                        vnorm[hb][r0_:r0_ + 64, :],
                                        vals[64:128, :], rbc[64:128, :], MUL)
                        return go

                    pending_norm = None
                    for hb in range(H // 2):
                        h0, h1 = 2 * hb, 2 * hb + 1

                        def qk_exp(skb):
                            st0 = mm_tile(SQ)
                            st1 = mm_tile(SQ)
                            for r0, st in ((0, st0), (64, st1)):
                                for j in range(2):
                                    nc.tensor.matmul(
                                        st[:, j * 512:(j + 1) * 512],
                                        kT[hb][r0:r0 + 64,
                                               skb * 128:(skb + 1) * 128],
                                        qT[hb][r0:r0 + 64,
                                               j * 512:(j + 1) * 512],
                                        start=True, stop=True)
                            pt0 = p_pool.tile([128, SQ], bf16, name="pTb")
                            pt1 = p_pool.tile([128, SQ], bf16, name="pTb")
                            # One head to ACT (exact exp), one to DVE
                            # (Schraudolph), alternating per skb.  Each
                            # engine then needs ~1.3us per skb < the PE's
                            # ~1.9us -> the PE streams gaplessly, keeping the
                            # HAM clock-gate at 8/8 (warm is only self-
                            # sustaining if the PE never waits: once HAM
                            # rethrottles it tends to stick cold).
                            if hb == 0 and skb < 3:
                                # ACT may still be draining projection
                                # copies right at the transition
                                pairs = ((None, None), (pt0, st0), (pt1, st1))
                            elif skb % 2 == 0:
                                pairs = ((pt0, st0), (pt1, st1), (None, None))
                            else:
                                pairs = ((pt1, st1), (pt0, st0), (None, None))
                            (a_pt, a_st), (d1_pt, d1_st), (d2_pt, d2_st) = \
                                pairs
                            if a_pt is not None:
                                nc.scalar.activation(a_pt[:, :], a_st[:, :],
                                                     EXP, scale=0.125)
                            for pt, st in ((d1_pt, d1_st), (d2_pt, d2_st)):
                                if pt is not None:
                                    nc.vector.tensor_scalar(
                                        pt.bitcast(i16)[:, :], st[:, :],
                                        SCHRAUD_A, SCHRAUD_B, MUL, ADD)
                            return (pt0, pt1)

                        def pv(skb, pts, vt0, vt1):
                            pt0, pt1 = pts
                            for h, vt, pt in ((h0, vt0, pt0), (h1, vt1, pt1)):
                                for j in range(2):
                                    nc.tensor.matmul(
                                        vt[:, j * 512:(j + 1) * 512],
                                        vp[skb][:, h * 128:(h + 1) * 128],
                                        pt[:, j * 512:(j + 1) * 512],
                                        start=(skb == 0),
                                        stop=(skb == SKB - 1))

                        # prologue: QK+exp of skbs 0-1, then the previous
                        # pair's deferred norm, then allocate this pair's vt
                        # tiles (allocation after the norm keeps the PSUM-
                        # slot dependency order correct).  Two skbs of QK
                        # give the PE enough work to cover the vals copies
                        # that free the vt slots.
                        # One-stage software pipeline in the PE FIFO:
                        # ... QK(n+1), PV(n), QK(n+2), PV(n+1) ... -- while
                        # PV(n) waits for exp(n), the PE is already running
                        # QK(n+1) instead of idling behind it in the queue.
                        # 3 QKs up front give the PE work to cover the
                        # deferred norm's copies at the pair boundary.
                        plg = [qk_exp(0), qk_exp(1), qk_exp(2)]
                        if pending_norm is not None:
                            pending_norm()
                        vt0 = vt_ps.tile([128, SQ], f32, name="valT")
                        vt1 = vt_ps.tile([128, SQ], f32, name="valT")
                        pv(0, plg[0], vt0, vt1)
                        pv(1, plg[1], vt0, vt1)
                        prev = plg[2]
                        for skb in range(3, SKB):
                            cur = qk_exp(skb)
                            pv(skb - 1, prev, vt0, vt1)
                            prev = cur
                        pv(SKB - 1, prev, vt0, vt1)
                        pending_norm = make_norm(hb, vt0, vt1)

                    # ---- output projection ----
                    # phase 1: open sqb0/sqb1 accumulators over kb 0..4 (only
                    # needs pairs 0-4) so the PE chews on them while the last
                    # pair's norm chain (emitted next) runs on ACT/DVE
                    def oproj_mms(op, sqb, kbs, first, final):
                        for nc2 in range(2):
                            n0, n1 = nc2 * 512, min(D, (nc2 + 1) * 512)
                            for kb in kbs:
                                nc.tensor.matmul(
                                    op[:, n0:n1],
                                    vnorm[kb][:, sqb * 128:(sqb + 1) * 128],
                                    woT[kb][:, n0:n1],
                                    start=(kb == 0 and first),
                                    stop=(kb == DB - 1 and final))

                    with tc.tile_pool(name="osb", bufs=3) as o_pool:
                        op01 = []
                        for sqb in range(2):
                            op = mm_tile(D)
                            oproj_mms(op, sqb, range(DB - 1), True, False)
                            op01.append(op)
                        pending_norm()
                        for sqb in range(SQB):
                            if sqb < 2:
                                op = op01[sqb]
                                oproj_mms(op, sqb, [DB - 1], False, True)
                            elif sqb % 2 == 0:
                                op = mm_tile(D)
                                oproj_mms(op, sqb, range(DB), True, True)
                            else:
                                op = vt_ps.tile([128, D], f32, name="valT",
                                                tag="valT",
                                                padded_shape=[128, SQ])
                                oproj_mms(op, sqb, range(DB), True, True)
                            ot = o_pool.tile([128, D], f32, name="osb")
                            nc.vector.tensor_copy(ot[:, :], op[:, :])
                            nc.sync.dma_start(
                                out=out[sqb * 128:(sqb + 1) * 128, :],
                                in_=ot[:, :])

    nc.compile()
    return nc


def _get_nc():
    if "nc" not in _cache:
        _cache["nc"] = _build_nc()
    return _cache["nc"]


def _host_fallback(x, y, mask, Wq, bq, Wkv, bkv, Wo, bo):
    Bb, Ss, _ = x.shape
    q = x @ Wq.T + bq
    kv = y @ Wkv.T + bkv
    q = q.reshape(Bb, Ss, H, Dh).transpose(0, 2, 1, 3)
    kv = kv.reshape(Bb, Ss, H, 2 * Dh).transpose(0, 2, 1, 3)
    k, v = kv[..., :Dh], kv[..., Dh:]
    scaled = np.einsum("bhqd,bhkd->bhqk", q, k) / np.sqrt(np.float32(Dh))
    scaled = scaled + mask
    scaled -= scaled.max(axis=-1, keepdims=True)
    e = np.exp(scaled)
    attn = e / e.sum(axis=-1, keepdims=True)
    values = np.einsum("bhqk,bhkd->bhqd", attn, v)
    values = values.transpose(0, 2, 1, 3).reshape(Bb, Ss, H * Dh)
    return (values @ Wo.T + bo).astype(np.float32)


def _run(inputs, trace=False, trace_cores=None):
    """Returns (full_output, BassKernelResults)."""
    from concourse.bass_utils import run_bass_kernel_spmd

    x = np.ascontiguousarray(np.asarray(inputs["x"], dtype=np.float32))
    y = np.ascontiguousarray(np.asarray(inputs["y"], dtype=np.float32))
    Wq = np.asarray(inputs["Wq"], dtype=np.float32)
    Wkv = np.asarray(inputs["Wkv"], dtype=np.float32)
    Wo = np.asarray(inputs["Wo"], dtype=np.float32)

    # Reference reshapes kv to [B,S,H,2*Dh]: per head, rows h*128..h*128+63 of
    # Wkv are the k-projection, rows h*128+64..h*128+127 the v-projection.
    k_rows = np.concatenate([np.arange(h * 128, h * 128 + Dh) for h in range(H)])
    v_rows = np.concatenate([np.arange(h * 128 + Dh, (h + 1) * 128)
                             for h in range(H)])
    import ml_dtypes
    bf16 = ml_dtypes.bfloat16
    WqT = np.ascontiguousarray(Wq.T)
    WkT = np.ascontiguousarray(Wkv[k_rows].T.astype(bf16))
    WvT = np.ascontiguousarray(Wkv[v_rows].T.astype(bf16))
    WoT = np.ascontiguousarray(Wo.T.astype(bf16))

    in_maps = []
    for c in range(N_CORES):
        b, half = c // 2, c % 2
        xTc = np.ascontiguousarray(x[b, half * SQ:(half + 1) * SQ, :].T)
        yTb = np.ascontiguousarray(y[b].T.astype(bf16))
        in_maps.append({"xT": xTc, "yT": yTb, "WqT": WqT, "WkT": WkT,
                        "WvT": WvT, "WoT": WoT})

    nc = _get_nc()
    res = run_bass_kernel_spmd(nc, in_maps, core_ids=list(range(N_CORES)),
                               trace=trace, trace_cores=trace_cores)
    out = np.empty((B, S, D), dtype=np.float32)
    for c in range(N_CORES):
        b, half = c // 2, c % 2
        out[b, half * SQ:(half + 1) * SQ, :] = res.results[c]["out"]
    return out, res


def kernel(**inputs) -> np.ndarray:
    mask = np.asarray(inputs["mask"], dtype=np.float32)
    bq = np.asarray(inputs["bq"], dtype=np.float32)
    bkv = np.asarray(inputs["bkv"], dtype=np.float32)
    bo = np.asarray(inputs["bo"], dtype=np.float32)
    if mask.any() or bq.any() or bkv.any() or bo.any():
        # Device kernel hardcodes zero mask/biases; stay correct regardless.
        return _host_fallback(
            np.asarray(inputs["x"], dtype=np.float32),
            np.asarray(inputs["y"], dtype=np.float32),
            mask, np.asarray(inputs["Wq"], dtype=np.float32), bq,
            np.asarray(inputs["Wkv"], dtype=np.float32), bkv,
            np.asarray(inputs["Wo"], dtype=np.float32), bo)
    out, _ = _run(inputs)
    return out

